# revision 19
# baseline (speedup 1.0000x reference)
# MoE kernel for Trainium2 (8 NeuronCores, expert-parallel).
#
# Strategy:
#  - Host: gate logits = x @ gate_w, top-2 + softmax, gather tokens per expert
#    (the "all-to-all by routed expert" from the sharding hint, done host-side
#    since we hold full inputs), pad each expert's token set to a common
#    capacity C (= max expert load, rounded to 4).
#  - Device (core e = expert e): h = gelu(xg^T-major GEMM w1) ; y = h GEMM w2.
#    Both GEMMs in bf16 on the PE array (1 cycle/row), fp32 PSUM accumulate.
#    Token dim rides the matmul free axis; D/dff ride partitions. Inputs are
#    pre-blocked on the host so every DMA chunk is contiguous in DRAM.
#  - Host: scatter-add wts * (y + b2[e]) back into the output.
#
# Perf notes (from NTFF profile analysis):
#  - The matmul stream itself runs gap-free at ~97% of the warm-clock roofline;
#    all recoverable time is the head (descriptor-issue serialization + cold
#    HAM clock) and the tail.
#  - DMA descriptor issue costs ~600ns per dma_start, serialized per engine.
#    Critical tensors (x tile 0 + w1 group 0) are issued on BOTH HW-DGE
#    engines (sync + scalar) in parallel, ahead of everything else.
#  - ~56 warmup matmuls on scratch SBUF run during the DMA wait so the PE's
#    HAM clock gate is already released (2.4 GHz) when real work starts.
#  - Capacity is exact (max expert load rounded to 4), not rounded to 128:
#    token tiles are [512..., rem-128, 128] so every matmul free dim >= 128.
import math
from contextlib import ExitStack

import ml_dtypes
import numpy as np

import concourse.bass as bass
import concourse.mybir as mybir
import concourse.tile as tile
from concourse.bass_utils import run_bass_kernel_spmd

D = 1024
DFF = 4096
E = 8
TOP_K = 2
P = 128
KD = D // P      # 8  contraction tiles for GEMM1
NF = DFF // P    # 32 dff tiles (GEMM1 out / GEMM2 contraction)
ND = D // P      # 8  GEMM2 out tiles
T_TILE = 512
# w1 f-chunks per DMA group: first groups small so the critical first wave
# (x tile 0 + w1 group 0) is light; f-tile f lives in group G_OF[f] at
# within-group index J_OF[f].
FGS = [4, 4, 4, 4, 4, 4, 4, 4]
NG = len(FGS)
FG_OFF = [sum(FGS[:i]) for i in range(NG)]
G_OF, J_OF = [], []
for _g, _fg in enumerate(FGS):
    for _j in range(_fg):
        G_OF.append(_g)
        J_OF.append(_j)
WJ = 8           # w2 f-strips per tile (16KB DMA rows)
NW = NF // WJ    # 4 w2 tiles

BF16 = mybir.dt.bfloat16
F32 = mybir.dt.float32
NP_BF16 = np.dtype(ml_dtypes.bfloat16)

_neff_cache = {}


def _t_tiles(C):
    """Token tiles: 512s first, then split the remainder so no tile < 128."""
    out, rem = [], C
    while rem >= 512 + 128:
        out.append(512)
        rem -= 512
    if rem > 512:
        out.append(rem - 128)
        out.append(128)
    elif rem:
        out.append(rem)
    return out


def _split_multiwait_json(bir_bytes: bytes) -> bytes:
    """The walrus build in this container rejects instructions carrying more
    than one sync wait (or update). Split extras onto adjacent single-wait
    EventSemaphore carriers on the same engine: program order on the engine
    preserves the semantics exactly."""
    import json as _json

    bir = _json.loads(bir_bytes)
    for fn in bir["functions"]:
        for blk in fn["blocks"]:
            insts = blk.get("instructions", [])
            out = []
            for inst in insts:
                si = inst.get("sync_info")
                if si:
                    waits = si.get("on_wait") or []
                    if len(waits) > 1:
                        for i, w in enumerate(waits[:-1]):
                            out.append({
                                "debug": inst.get("debug", 0),
                                "engine": inst["engine"],
                                "ins": [],
                                "name": f"{inst['name']}_w{i}",
                                "opcode": "EventSemaphore",
                                "outs": [],
                                "sync_info": {"on_update": [], "on_wait": [w]},
                            })
                        si["on_wait"] = [waits[-1]]
                out.append(inst)
                if si:
                    ups = si.get("on_update") or []
                    if len(ups) > 1:
                        for i, u in enumerate(ups[1:]):
                            out.append({
                                "debug": inst.get("debug", 0),
                                "engine": inst["engine"],
                                "ins": [],
                                "name": f"{inst['name']}_u{i}",
                                "opcode": "EventSemaphore",
                                "outs": [],
                                "sync_info": {"on_update": [u], "on_wait": []},
                            })
                        si["on_update"] = [ups[0]]
            blk["instructions"] = out
    return _json.dumps(bir).encode()


def _patch_to_json(nc: bass.Bass) -> bass.Bass:
    orig = nc.to_json_bytes
    nc.to_json_bytes = lambda: _split_multiwait_json(orig())
    return nc


def _build_bass(tiles) -> bass.Bass:
    """One expert's MLP in transposed layouts (token dim = free axis).

    DRAM input layouts (pre-blocked on host so each partition's span is
    contiguous and large; DMAs are band-split across partitions onto
    parallel HW-DGE queues):
      xs : flat bf16; tile ti's block is [P, KD*tsz] p-major, rows KD*tsz*2 B
      w1x: [NG * P, KD * FG * P] bf16; row g*P+p holds w1 f-group g (8KB)
      w2x: [NW * P, WJ * D] bf16; row j*P+p holds w2 f-strips j*WJ.. (16KB)
      b1 : [DFF] f32
    Output:
      y  : [ND * n_t * P, T_TILE] bf16, block (dd, ti) at rows (dd*n_t+ti)*P
    """
    nc = bass.Bass()
    n_t = len(tiles)
    x_elems = P * KD * sum(tiles)
    xs_h = nc.dram_tensor("xs", [x_elems], BF16, kind="ExternalInput")
    w1_h = nc.dram_tensor("w1", [P * KD * DFF], BF16, kind="ExternalInput")
    b1_h = nc.dram_tensor("b1", [DFF], F32, kind="ExternalInput")
    w2_h = nc.dram_tensor("w2", [NW * P, WJ * D], BF16, kind="ExternalInput")
    y_h = nc.dram_tensor("y", [ND * n_t * P, T_TILE], BF16, kind="ExternalOutput")

    with ExitStack() as ctx:
        tc = ctx.enter_context(tile.TileContext(nc))
        spool = ctx.enter_context(tc.tile_pool(name="s", bufs=1))
        wpool = ctx.enter_context(tc.tile_pool(name="w", bufs=1))
        xpool = ctx.enter_context(tc.tile_pool(name="x", bufs=1))
        hpool = ctx.enter_context(tc.tile_pool(name="h", bufs=1))
        bpool = ctx.enter_context(tc.tile_pool(name="b", bufs=1))
        ypool = ctx.enter_context(tc.tile_pool(name="y", bufs=3))
        ps1 = ctx.enter_context(tc.tile_pool(name="ps1", bufs=3, space="PSUM"))
        ps2 = ctx.enter_context(tc.tile_pool(name="ps2", bufs=1, space="PSUM"))

        # --- PE warmup: run scratch matmuls during the initial DMA wait so
        # the HAM clock gate opens (K=8/8) before the first real matmul.
        scr = spool.tile([P, P + T_TILE], BF16, name="scr")
        nc.gpsimd.memset(scr[:], 1.0)
        # Back-to-back writes to one PSUM bank self-serialize at ~390ns/MM
        # (drain before has_written clear), which is what we want: a steady
        # ~12.5us of PE busy that ends just as the critical DMA lands.
        wps = ps1.tile([P, T_TILE], F32, tag="ps1", name="wps")
        for _ in range(32):
            nc.tensor.matmul(wps[:], scr[:, :P], scr[:, P:],
                             start=True, stop=True)

        # --- DMA plan. One dma_start's rows already fan out over all 16 DMA
        # engines of the bank, and the framework throttles at 4 outstanding
        # DMAs (rotating slot semaphores, wait-on-reuse). So: emit ~0.5MB
        # half-tensor pieces on the sync bank in priority order — the slot
        # rotation then paces later waves behind earlier ones, keeping the
        # critical first wave (x tile 0 + w1 group 0) alone on the wire.
        def w1_tile(g, npc, eng=None):
            fg = FGS[g]
            t = wpool.tile([P, KD, fg * P], BF16, tag=f"w1_{g}", name=f"w1_{g}")
            off = P * KD * FG_OFF[g] * P
            bp = P // npc
            blk = KD * fg * P
            for b in range(npc):
                o = off + b * bp * blk
                (eng or nc.sync).dma_start(
                    t[b * bp:(b + 1) * bp, :, :],
                    w1_h[o:o + bp * blk].rearrange(
                        "(p kd m) -> p kd m", p=bp, kd=KD),
                )
            return t

        def x_tile(ti, off, tsz, npc):
            t = xpool.tile([P, KD, T_TILE], BF16, tag=f"x{ti}", name=f"x{ti}")
            bp = P // npc
            blk = KD * tsz
            for b in range(npc):
                o = off + b * bp * blk
                nc.sync.dma_start(
                    t[b * bp:(b + 1) * bp, :, :tsz],
                    xs_h[o:o + bp * blk].rearrange(
                        "(p kd c) -> p kd c", p=bp, kd=KD),
                )
            return t

        def w2_tile(j):
            t = wpool.tile([P, WJ * D], BF16, tag=f"w2_{j}", name=f"w2_{j}")
            for b in range(2):
                r0 = j * P + b * 64
                nc.sync.dma_start(t[b * 64:(b + 1) * 64, :], w2_h[r0:r0 + 64, :])
            return t

        x_off = [P * KD * sum(tiles[:i]) for i in range(n_t)]
        x_t = [None] * n_t
        w1_t = [None] * NG
        w2_t = [None] * NW
        # Critical wave: x0 halves on the sync bank, w1 g0 halves on the
        # scalar bank. Early DMA is row-count bound (~600-800ns/row/engine,
        # descriptor fetches pipeline across queues), so spreading the 256
        # critical rows over both banks' queues shortens the head.
        x_t[0] = x_tile(0, x_off[0], tiles[0], 2)
        w1_t[0] = w1_tile(0, 2, eng=nc.scalar)
        # then the rest of w1 (needed at ~6.8us per 4-f group during GEMM1 of
        # tile 0), then w2 (GEMM2 of tile 0 consumes strip j over a ~27us
        # window thanks to the f-outer loop), then x1/x2 (needed much later).
        for g in range(1, NG):
            w1_t[g] = w1_tile(g, 1)
        for j in range(NW):
            w2_t[j] = w2_tile(j)
        if n_t > 1:
            x_t[1] = x_tile(1, x_off[1], tiles[1], 2)
        for ti in range(2, n_t):
            x_t[ti] = x_tile(ti, x_off[ti], tiles[ti], 1)
        # b1 rides the (otherwise idle) scalar HW-DGE bank so it beats the
        # first gelu; the ACT-engine copy funnels it into scalar program order.
        b1_raw = bpool.tile([P, NF], F32)
        nc.scalar.dma_start(b1_raw[:], b1_h[:].rearrange("(f p) -> p f", p=P))
        b1_t = bpool.tile([P, NF], F32)
        nc.scalar.copy(b1_t[:], b1_raw[:])

        gelu = mybir.ActivationFunctionType.Gelu

        def w2_ap(f, dd):
            return w2_t[f // WJ][:, (f % WJ) * D + dd * P:
                                 (f % WJ) * D + (dd + 1) * P]

        def y_out(dd, ti, tsz, pt2):
            y_t = ypool.tile([P, T_TILE], BF16, tag="y", name="yt")
            nc.vector.tensor_copy(y_t[:, :tsz], pt2[:, :tsz])
            r0 = (dd * n_t + ti) * P
            nc.sync.dma_start(y_h[r0:r0 + P, :tsz], y_t[:, :tsz])

        for ti, tsz in enumerate(tiles):
            h_t = [hpool.tile([P, T_TILE], BF16, tag=f"h{f}", name=f"h{f}")
                   for f in range(NF)]
            for f in range(NF):
                pt = ps1.tile([P, T_TILE], F32, tag="ps1", name="pt1")
                for k in range(KD):
                    nc.tensor.matmul(
                        pt[:, :tsz],
                        w1_t[G_OF[f]][:, k, J_OF[f] * P:(J_OF[f] + 1) * P],
                        x_t[ti][:, k, :tsz],
                        start=(k == 0),
                        stop=(k == KD - 1),
                    )
                nc.scalar.activation(
                    h_t[f][:, :tsz], pt[:, :tsz], gelu, bias=b1_t[:, f:f + 1]
                )
            if ti < n_t - 1:
                # f-outer, dd-inner in halves of 4 PSUM banks: w2 strip j is
                # then consumed over a ~27us window instead of the first 7us
                # of GEMM2, so tile-0 GEMM2 doesn't stall on w2 arrival.
                for half in range(2):
                    pts = [ps2.tile([P, T_TILE], F32, tag=f"ps2_{i}",
                                    name=f"pt2_{i}") for i in range(4)]
                    for f in range(NF):
                        for i in range(4):
                            nc.tensor.matmul(
                                pts[i][:, :tsz],
                                w2_ap(f, half * 4 + i),
                                h_t[f][:, :tsz],
                                start=(f == 0),
                                stop=(f == NF - 1),
                            )
                    for i in range(4):
                        y_out(half * 4 + i, ti, tsz, pts[i])
            else:
                # last tile: dd-outer so y write-backs stream out during the
                # loop and only one copy+DMA lands in the kernel tail.
                for dd in range(ND):
                    pt2 = ps2.tile([P, T_TILE], F32, tag=f"ps2_{dd % 4}",
                                   name="pt2")
                    for f in range(NF):
                        nc.tensor.matmul(
                            pt2[:, :tsz],
                            w2_ap(f, dd),
                            h_t[f][:, :tsz],
                            start=(f == 0),
                            stop=(f == NF - 1),
                        )
                    y_out(dd, ti, tsz, pt2)
    return _patch_to_json(nc)


def _route(xf: np.ndarray, gate_w: np.ndarray):
    """Top-2 gating identical to the reference (argmax ties -> lower index)."""
    N = xf.shape[0]
    logits = xf @ gate_w  # (N, E) f32
    rows = np.arange(N)
    i1 = logits.argmax(1)
    v1 = logits[rows, i1]
    masked = logits.copy()
    masked[rows, i1] = -np.inf
    i2 = masked.argmax(1)
    v2 = masked[rows, i2]
    # softmax over the two selected logits (v1 >= v2)
    e = np.exp((v2 - v1).astype(np.float32))
    wt1 = (1.0 / (1.0 + e)).astype(np.float32)
    wt2 = (e / (1.0 + e)).astype(np.float32)
    idx_e, wts_e = [], []
    for ex in range(E):
        s1 = np.nonzero(i1 == ex)[0]
        s2 = np.nonzero(i2 == ex)[0]
        idx_e.append(np.concatenate([s1, s2]))
        wts_e.append(np.concatenate([wt1[s1], wt2[s2]]).astype(np.float32))
    return idx_e, wts_e


def kernel(x, gate_w, w1, b1, w2, b2, _trace=False):
    B, T, D_ = x.shape
    N = B * T
    xf = np.ascontiguousarray(x.reshape(N, D_).astype(np.float32))
    idx_e, wts_e = _route(xf, gate_w.astype(np.float32))
    cnts = [len(i) for i in idx_e]
    C = max(P, int(math.ceil(max(cnts) / 4)) * 4)
    tiles = _t_tiles(C)
    n_t = len(tiles)

    key = tuple(tiles)
    if key in _neff_cache:
        nc = _neff_cache[key]
    else:
        nc = _build_bass(tiles)
        _neff_cache[key] = nc

    in_maps = []
    for ex in range(E):
        cnt = cnts[ex]
        xg = np.zeros((C, D), np.float32)
        if cnt:
            xg[:cnt] = xf[idx_e[ex]]
        xgT = np.ascontiguousarray(xg.T).astype(NP_BF16)  # (D, C)
        # per tile ti: block [P, KD*tsz] with row p = concat_kd xgT[kd*P+p, t0:t0+tsz]
        parts = []
        t0 = 0
        for tsz in tiles:
            blk = (
                xgT[:, t0:t0 + tsz]
                .reshape(KD, P, tsz)
                .transpose(1, 0, 2)
                .reshape(P, KD * tsz)
            )
            parts.append(blk.ravel())
            t0 += tsz
        xs = np.concatenate(parts)
        # w1x: flat, group g block [P, KD*FGS[g]*P]: row p = concat_kd of
        # w1[kd*P+p, FG_OFF[g]*P : (FG_OFF[g]+FGS[g])*P]
        w1b = w1[ex].astype(NP_BF16)
        w1parts = []
        for g in range(NG):
            c0 = FG_OFF[g] * P
            c1 = c0 + FGS[g] * P
            blk = (
                w1b[:, c0:c1]
                .reshape(KD, P, FGS[g] * P)
                .transpose(1, 0, 2)
                .reshape(P, KD * FGS[g] * P)
            )
            w1parts.append(blk.ravel())
        w1x = np.concatenate(w1parts)
        # w2x[j*P + p, fi*D + d] = w2[(j*WJ+fi)*P + p, d]
        w2x = (
            w2[ex]
            .reshape(NW, WJ, P, D)
            .transpose(0, 2, 1, 3)
            .reshape(NW * P, WJ * D)
        )
        in_maps.append({
            "xs": np.ascontiguousarray(xs),
            "w1": np.ascontiguousarray(w1x),
            "b1": np.ascontiguousarray(b1[ex]).astype(np.float32),
            "w2": np.ascontiguousarray(w2x).astype(NP_BF16),
        })

    res = run_bass_kernel_spmd(nc, in_maps, core_ids=list(range(E)), trace=_trace)
    if _trace:
        print(f"HW exec time: {res.exec_time_ns} ns")

    out = np.zeros((N, D), np.float32)
    for ex in range(E):
        cnt = cnts[ex]
        if not cnt:
            continue
        yb = res.results[ex]["y"]  # [ND*n_t*P, T_TILE] bf16
        yb = np.asarray(yb).astype(np.float32)
        yt = np.empty((D, cnt), np.float32)
        t0 = 0
        for ti, tsz in enumerate(tiles):
            if t0 >= cnt:
                break
            w = min(tsz, cnt - t0)
            for dd in range(ND):
                r0 = (dd * n_t + ti) * P
                yt[dd * P:(dd + 1) * P, t0:t0 + w] = yb[r0:r0 + P, :w]
            t0 += tsz
        yv = yt.T + b2[ex][None, :].astype(np.float32)
        out[idx_e[ex]] += wts_e[ex][:, None] * yv
    return out.reshape(B, T, D_)


# revision 22
# speedup vs baseline: 1.0754x; 1.0754x over previous
# MoE kernel for Trainium2 (8 NeuronCores, expert-parallel).
#
# Strategy:
#  - Host: gate logits = x @ gate_w, top-2 + softmax, gather tokens per expert
#    (the "all-to-all by routed expert" from the sharding hint, done host-side
#    since we hold full inputs), pad each expert's token set to a common
#    capacity C (= max expert load, rounded to 4).
#  - Device (core e = expert e): h = gelu(xg^T-major GEMM w1) ; y = h GEMM w2.
#    Both GEMMs in bf16 on the PE array (1 cycle/row), fp32 PSUM accumulate.
#    Token dim rides the matmul free axis; D/dff ride partitions. Inputs are
#    pre-blocked on the host so every DMA chunk is contiguous in DRAM.
#  - Host: scatter-add wts * (y + b2[e]) back into the output.
#
# Perf notes (from NTFF profile analysis):
#  - The matmul stream itself runs gap-free at ~97% of the warm-clock roofline;
#    all recoverable time is the head (descriptor-issue serialization + cold
#    HAM clock) and the tail.
#  - DMA descriptor issue costs ~600ns per dma_start, serialized per engine.
#    Critical tensors (x tile 0 + w1 group 0) are issued on BOTH HW-DGE
#    engines (sync + scalar) in parallel, ahead of everything else.
#  - ~56 warmup matmuls on scratch SBUF run during the DMA wait so the PE's
#    HAM clock gate is already released (2.4 GHz) when real work starts.
#  - Capacity is exact (max expert load rounded to 4), not rounded to 128:
#    token tiles are [512..., rem-128, 128] so every matmul free dim >= 128.
import math
from contextlib import ExitStack

import ml_dtypes
import numpy as np

import concourse.bass as bass
import concourse.mybir as mybir
import concourse.tile as tile
from concourse.bass_utils import run_bass_kernel_spmd

D = 1024
DFF = 4096
E = 8
TOP_K = 2
P = 128
KD = D // P      # 8  contraction tiles for GEMM1
NF = DFF // P    # 32 dff tiles (GEMM1 out / GEMM2 contraction)
ND = D // P      # 8  GEMM2 out tiles
T_TILE = 512
# w1 f-chunks per DMA group: first groups small so the critical first wave
# (x tile 0 + w1 group 0) is light; f-tile f lives in group G_OF[f] at
# within-group index J_OF[f].
FGS = [4, 4, 4, 4, 4, 4, 4, 4]
NG = len(FGS)
FG_OFF = [sum(FGS[:i]) for i in range(NG)]
G_OF, J_OF = [], []
for _g, _fg in enumerate(FGS):
    for _j in range(_fg):
        G_OF.append(_g)
        J_OF.append(_j)
WJ = 8           # w2 f-strips per tile (16KB DMA rows)
NW = NF // WJ    # 4 w2 tiles

BF16 = mybir.dt.bfloat16
F32 = mybir.dt.float32
NP_BF16 = np.dtype(ml_dtypes.bfloat16)

_neff_cache = {}


def _t_tiles(C):
    """Token tiles: 512s first, then split the remainder so no tile < 128."""
    out, rem = [], C
    while rem >= 512 + 128:
        out.append(512)
        rem -= 512
    if rem > 512:
        out.append(rem - 128)
        out.append(128)
    elif rem:
        out.append(rem)
    return out


def _split_multiwait_json(bir_bytes: bytes) -> bytes:
    """The walrus build in this container rejects instructions carrying more
    than one sync wait (or update). Split extras onto adjacent single-wait
    EventSemaphore carriers on the same engine: program order on the engine
    preserves the semantics exactly."""
    import json as _json

    bir = _json.loads(bir_bytes)
    for fn in bir["functions"]:
        for blk in fn["blocks"]:
            insts = blk.get("instructions", [])
            out = []
            for inst in insts:
                si = inst.get("sync_info")
                if si:
                    waits = si.get("on_wait") or []
                    if len(waits) > 1:
                        for i, w in enumerate(waits[:-1]):
                            out.append({
                                "debug": inst.get("debug", 0),
                                "engine": inst["engine"],
                                "ins": [],
                                "name": f"{inst['name']}_w{i}",
                                "opcode": "EventSemaphore",
                                "outs": [],
                                "sync_info": {"on_update": [], "on_wait": [w]},
                            })
                        si["on_wait"] = [waits[-1]]
                out.append(inst)
                if si:
                    ups = si.get("on_update") or []
                    if len(ups) > 1:
                        for i, u in enumerate(ups[1:]):
                            out.append({
                                "debug": inst.get("debug", 0),
                                "engine": inst["engine"],
                                "ins": [],
                                "name": f"{inst['name']}_u{i}",
                                "opcode": "EventSemaphore",
                                "outs": [],
                                "sync_info": {"on_update": [u], "on_wait": []},
                            })
                        si["on_update"] = [ups[0]]
            blk["instructions"] = out
    return _json.dumps(bir).encode()


def _patch_to_json(nc: bass.Bass) -> bass.Bass:
    orig = nc.to_json_bytes
    nc.to_json_bytes = lambda: _split_multiwait_json(orig())
    return nc


def _build_bass(tiles) -> bass.Bass:
    """One expert's MLP in transposed layouts (token dim = free axis).

    DRAM input layouts (pre-blocked on host so each partition's span is
    contiguous and large; DMAs are band-split across partitions onto
    parallel HW-DGE queues):
      xs : flat bf16; tile ti's block is [P, KD*tsz] p-major, rows KD*tsz*2 B
      w1x: [NG * P, KD * FG * P] bf16; row g*P+p holds w1 f-group g (8KB)
      w2x: [NW * P, WJ * D] bf16; row j*P+p holds w2 f-strips j*WJ.. (16KB)
      b1 : [DFF] f32
    Output:
      y  : [ND * n_t * P, T_TILE] bf16, block (dd, ti) at rows (dd*n_t+ti)*P
    """
    nc = bass.Bass()
    n_t = len(tiles)
    x_elems = P * KD * sum(tiles)
    xs_h = nc.dram_tensor("xs", [x_elems], BF16, kind="ExternalInput")
    w1_h = nc.dram_tensor("w1", [P * KD * DFF], BF16, kind="ExternalInput")
    b1_h = nc.dram_tensor("b1", [DFF], F32, kind="ExternalInput")
    w2_h = nc.dram_tensor("w2", [NW * P, WJ * D], BF16, kind="ExternalInput")
    y_h = nc.dram_tensor("y", [ND * n_t * P, T_TILE], BF16, kind="ExternalOutput")

    with ExitStack() as ctx:
        tc = ctx.enter_context(tile.TileContext(nc))
        spool = ctx.enter_context(tc.tile_pool(name="s", bufs=1))
        wpool = ctx.enter_context(tc.tile_pool(name="w", bufs=1))
        xpool = ctx.enter_context(tc.tile_pool(name="x", bufs=1))
        hpool = ctx.enter_context(tc.tile_pool(name="h", bufs=1))
        bpool = ctx.enter_context(tc.tile_pool(name="b", bufs=1))
        ypool = ctx.enter_context(tc.tile_pool(name="y", bufs=3))
        ps1 = ctx.enter_context(tc.tile_pool(name="ps1", bufs=3, space="PSUM"))
        ps2 = ctx.enter_context(tc.tile_pool(name="ps2", bufs=1, space="PSUM"))

        # --- PE warmup: run scratch matmuls during the initial DMA wait so
        # the HAM clock gate opens (K=8/8) before the first real matmul.
        scr = spool.tile([P, P + T_TILE], BF16, name="scr")
        nc.gpsimd.memset(scr[:], 1.0)
        # Back-to-back writes to one PSUM bank self-serialize at ~390ns/MM
        # (drain before has_written clear), which is what we want: a steady
        # ~12.5us of PE busy that ends just as the critical DMA lands.
        wps = ps1.tile([P, T_TILE], F32, tag="ps1", name="wps")
        for _ in range(40):
            nc.tensor.matmul(wps[:], scr[:, :P], scr[:, P:],
                             start=True, stop=True)

        # --- DMA plan. One dma_start's rows already fan out over all 16 DMA
        # engines of the bank, and the framework throttles at 4 outstanding
        # DMAs (rotating slot semaphores, wait-on-reuse). So: emit ~0.5MB
        # half-tensor pieces on the sync bank in priority order — the slot
        # rotation then paces later waves behind earlier ones, keeping the
        # critical first wave (x tile 0 + w1 group 0) alone on the wire.
        def w1_tile(g, npc, eng=None):
            fg = FGS[g]
            t = wpool.tile([P, KD, fg * P], BF16, tag=f"w1_{g}", name=f"w1_{g}")
            off = P * KD * FG_OFF[g] * P
            bp = P // npc
            blk = KD * fg * P
            for b in range(npc):
                o = off + b * bp * blk
                (eng or nc.sync).dma_start(
                    t[b * bp:(b + 1) * bp, :, :],
                    w1_h[o:o + bp * blk].rearrange(
                        "(p kd m) -> p kd m", p=bp, kd=KD),
                )
            return t

        def x_tile(ti, off, tsz, npc):
            t = xpool.tile([P, KD, T_TILE], BF16, tag=f"x{ti}", name=f"x{ti}")
            bp = P // npc
            blk = KD * tsz
            for b in range(npc):
                o = off + b * bp * blk
                nc.sync.dma_start(
                    t[b * bp:(b + 1) * bp, :, :tsz],
                    xs_h[o:o + bp * blk].rearrange(
                        "(p kd c) -> p kd c", p=bp, kd=KD),
                )
            return t

        def w2_tile(j):
            t = wpool.tile([P, WJ * D], BF16, tag=f"w2_{j}", name=f"w2_{j}")
            for b in range(2):
                r0 = j * P + b * 64
                nc.sync.dma_start(t[b * 64:(b + 1) * 64, :], w2_h[r0:r0 + 64, :])
            return t

        x_off = [P * KD * sum(tiles[:i]) for i in range(n_t)]
        x_t = [None] * n_t
        w1_t = [None] * NG
        w2_t = [None] * NW
        # Critical wave: x0 halves on the sync bank, w1 g0 halves on the
        # scalar bank. Early DMA is row-count bound (~600-800ns/row/engine,
        # descriptor fetches pipeline across queues), so spreading the 256
        # critical rows over both banks' queues shortens the head.
        x_t[0] = x_tile(0, x_off[0], tiles[0], 2)
        w1_t[0] = w1_tile(0, 2, eng=nc.scalar)
        # then the rest of w1 (needed at ~6.8us per 4-f group during GEMM1 of
        # tile 0), then w2 (GEMM2 of tile 0 consumes strip j over a ~27us
        # window thanks to the f-outer loop), then x1/x2 (needed much later).
        for g in range(1, NG):
            w1_t[g] = w1_tile(g, 1)
        for j in range(NW):
            w2_t[j] = w2_tile(j)
        if n_t > 1:
            x_t[1] = x_tile(1, x_off[1], tiles[1], 2)
        for ti in range(2, n_t):
            x_t[ti] = x_tile(ti, x_off[ti], tiles[ti], 1)
        # b1 rides the scalar HW-DGE bank so it beats the first gelu. It is
        # pre-transposed on the host to [P, NF] so this DMA is 128 contiguous
        # 128B rows — a strided elementwise gather here floods the shared DMA
        # engines with 4-byte packets and starves the weight streams.
        b1_t = bpool.tile([P, NF], F32)
        nc.scalar.dma_start(b1_t[:], b1_h[:].rearrange("(p f) -> p f", p=P))

        gelu = mybir.ActivationFunctionType.Gelu

        def w2_ap(f, dd):
            return w2_t[f // WJ][:, (f % WJ) * D + dd * P:
                                 (f % WJ) * D + (dd + 1) * P]

        def y_out(dd, ti, tsz, pt2):
            y_t = ypool.tile([P, T_TILE], BF16, tag="y", name="yt")
            nc.vector.tensor_copy(y_t[:, :tsz], pt2[:, :tsz])
            r0 = (dd * n_t + ti) * P
            nc.sync.dma_start(y_h[r0:r0 + P, :tsz], y_t[:, :tsz])

        for ti, tsz in enumerate(tiles):
            h_t = [hpool.tile([P, T_TILE], BF16, tag=f"h{f}", name=f"h{f}")
                   for f in range(NF)]
            for f in range(NF):
                pt = ps1.tile([P, T_TILE], F32, tag="ps1", name="pt1")
                for k in range(KD):
                    nc.tensor.matmul(
                        pt[:, :tsz],
                        w1_t[G_OF[f]][:, k, J_OF[f] * P:(J_OF[f] + 1) * P],
                        x_t[ti][:, k, :tsz],
                        start=(k == 0),
                        stop=(k == KD - 1),
                    )
                nc.scalar.activation(
                    h_t[f][:, :tsz], pt[:, :tsz], gelu, bias=b1_t[:, f:f + 1]
                )
            if ti < n_t - 1:
                # f-outer, dd-inner in halves of 4 PSUM banks: w2 strip j is
                # then consumed over a ~27us window instead of the first 7us
                # of GEMM2, so tile-0 GEMM2 doesn't stall on w2 arrival.
                for half in range(2):
                    pts = [ps2.tile([P, T_TILE], F32, tag=f"ps2_{i}",
                                    name=f"pt2_{i}") for i in range(4)]
                    for f in range(NF):
                        for i in range(4):
                            nc.tensor.matmul(
                                pts[i][:, :tsz],
                                w2_ap(f, half * 4 + i),
                                h_t[f][:, :tsz],
                                start=(f == 0),
                                stop=(f == NF - 1),
                            )
                    for i in range(4):
                        y_out(half * 4 + i, ti, tsz, pts[i])
            else:
                # last tile: dd-outer so y write-backs stream out during the
                # loop and only one copy+DMA lands in the kernel tail.
                for dd in range(ND):
                    pt2 = ps2.tile([P, T_TILE], F32, tag=f"ps2_{dd % 4}",
                                   name="pt2")
                    for f in range(NF):
                        nc.tensor.matmul(
                            pt2[:, :tsz],
                            w2_ap(f, dd),
                            h_t[f][:, :tsz],
                            start=(f == 0),
                            stop=(f == NF - 1),
                        )
                    y_out(dd, ti, tsz, pt2)
    return _patch_to_json(nc)


def _route(xf: np.ndarray, gate_w: np.ndarray):
    """Top-2 gating identical to the reference (argmax ties -> lower index)."""
    N = xf.shape[0]
    logits = xf @ gate_w  # (N, E) f32
    rows = np.arange(N)
    i1 = logits.argmax(1)
    v1 = logits[rows, i1]
    masked = logits.copy()
    masked[rows, i1] = -np.inf
    i2 = masked.argmax(1)
    v2 = masked[rows, i2]
    # softmax over the two selected logits (v1 >= v2)
    e = np.exp((v2 - v1).astype(np.float32))
    wt1 = (1.0 / (1.0 + e)).astype(np.float32)
    wt2 = (e / (1.0 + e)).astype(np.float32)
    idx_e, wts_e = [], []
    for ex in range(E):
        s1 = np.nonzero(i1 == ex)[0]
        s2 = np.nonzero(i2 == ex)[0]
        idx_e.append(np.concatenate([s1, s2]))
        wts_e.append(np.concatenate([wt1[s1], wt2[s2]]).astype(np.float32))
    return idx_e, wts_e


def kernel(x, gate_w, w1, b1, w2, b2, _trace=False):
    B, T, D_ = x.shape
    N = B * T
    xf = np.ascontiguousarray(x.reshape(N, D_).astype(np.float32))
    idx_e, wts_e = _route(xf, gate_w.astype(np.float32))
    cnts = [len(i) for i in idx_e]
    C = max(P, int(math.ceil(max(cnts) / 4)) * 4)
    tiles = _t_tiles(C)
    n_t = len(tiles)

    key = tuple(tiles)
    if key in _neff_cache:
        nc = _neff_cache[key]
    else:
        nc = _build_bass(tiles)
        _neff_cache[key] = nc

    in_maps = []
    for ex in range(E):
        cnt = cnts[ex]
        xg = np.zeros((C, D), np.float32)
        if cnt:
            xg[:cnt] = xf[idx_e[ex]]
        xgT = np.ascontiguousarray(xg.T).astype(NP_BF16)  # (D, C)
        # per tile ti: block [P, KD*tsz] with row p = concat_kd xgT[kd*P+p, t0:t0+tsz]
        parts = []
        t0 = 0
        for tsz in tiles:
            blk = (
                xgT[:, t0:t0 + tsz]
                .reshape(KD, P, tsz)
                .transpose(1, 0, 2)
                .reshape(P, KD * tsz)
            )
            parts.append(blk.ravel())
            t0 += tsz
        xs = np.concatenate(parts)
        # w1x: flat, group g block [P, KD*FGS[g]*P]: row p = concat_kd of
        # w1[kd*P+p, FG_OFF[g]*P : (FG_OFF[g]+FGS[g])*P]
        w1b = w1[ex].astype(NP_BF16)
        w1parts = []
        for g in range(NG):
            c0 = FG_OFF[g] * P
            c1 = c0 + FGS[g] * P
            blk = (
                w1b[:, c0:c1]
                .reshape(KD, P, FGS[g] * P)
                .transpose(1, 0, 2)
                .reshape(P, KD * FGS[g] * P)
            )
            w1parts.append(blk.ravel())
        w1x = np.concatenate(w1parts)
        # w2x[j*P + p, fi*D + d] = w2[(j*WJ+fi)*P + p, d]
        w2x = (
            w2[ex]
            .reshape(NW, WJ, P, D)
            .transpose(0, 2, 1, 3)
            .reshape(NW * P, WJ * D)
        )
        in_maps.append({
            "xs": np.ascontiguousarray(xs),
            "w1": np.ascontiguousarray(w1x),
            "b1": np.ascontiguousarray(
                b1[ex].astype(np.float32).reshape(NF, P).T).ravel(),
            "w2": np.ascontiguousarray(w2x).astype(NP_BF16),
        })

    res = run_bass_kernel_spmd(nc, in_maps, core_ids=list(range(E)), trace=_trace)
    if _trace:
        print(f"HW exec time: {res.exec_time_ns} ns")

    out = np.zeros((N, D), np.float32)
    for ex in range(E):
        cnt = cnts[ex]
        if not cnt:
            continue
        yb = res.results[ex]["y"]  # [ND*n_t*P, T_TILE] bf16
        yb = np.asarray(yb).astype(np.float32)
        yt = np.empty((D, cnt), np.float32)
        t0 = 0
        for ti, tsz in enumerate(tiles):
            if t0 >= cnt:
                break
            w = min(tsz, cnt - t0)
            for dd in range(ND):
                r0 = (dd * n_t + ti) * P
                yt[dd * P:(dd + 1) * P, t0:t0 + w] = yb[r0:r0 + P, :w]
            t0 += tsz
        yv = yt.T + b2[ex][None, :].astype(np.float32)
        out[idx_e[ex]] += wts_e[ex][:, None] * yv
    return out.reshape(B, T, D_)


# revision 24
# speedup vs baseline: 1.0773x; 1.0018x over previous
# MoE kernel for Trainium2 (8 NeuronCores, expert-parallel).
#
# Strategy:
#  - Host: gate logits = x @ gate_w, top-2 + softmax, gather tokens per expert
#    (the "all-to-all by routed expert" from the sharding hint, done host-side
#    since we hold full inputs), pad each expert's token set to a common
#    capacity C (= max expert load, rounded to 4).
#  - Device (core e = expert e): h = gelu(xg^T-major GEMM w1) ; y = h GEMM w2.
#    Both GEMMs in bf16 on the PE array (1 cycle/row), fp32 PSUM accumulate.
#    Token dim rides the matmul free axis; D/dff ride partitions. Inputs are
#    pre-blocked on the host so every DMA chunk is contiguous in DRAM.
#  - Host: scatter-add wts * (y + b2[e]) back into the output.
#
# Perf notes (from NTFF profile analysis; ~262us vs 282.5us baseline):
#  - The matmul stream runs gap-free at the warm-clock bf16 roofline; the
#    recoverable time was the head (DMA descriptor-issue serialization, cold
#    HAM clock, padded capacity) and stalls on late weight arrival.
#  - One dma_start's rows fan out over all 16 DMA engines; the framework
#    throttles at 4 outstanding DMAs per HW-DGE bank (rotating slot
#    semaphores). Early DMA is row-count bound (~600-800ns/row/engine), so
#    the critical wave is exactly the 256 rows of x tile 0 + w1 group 0,
#    split across the sync and scalar banks, everything else queued behind.
#  - Warmup matmuls on scratch SBUF span the DMA wait so the PE's HAM clock
#    gate is released (2.4 GHz) when real work starts, with no >3.4us idle
#    gap (which would re-throttle).
#  - Capacity is exact (max expert load rounded to 4), not rounded to 128:
#    token tiles are [512..., rem-128, 128] so every matmul free dim >= 128.
#  - GEMM2 runs f-outer/dd-inner in two 4-PSUM-bank halves so w2 strips are
#    consumed over ~27us (matches arrival); the last tile is dd-outer so only
#    one y copy+DMA lands in the kernel tail.
#  - Known run-to-run confound: the chip sometimes executes at 2.0GHz (P0
#    power state) instead of 2.4GHz, inflating everything ~20%.
import math
from contextlib import ExitStack

import ml_dtypes
import numpy as np

import concourse.bass as bass
import concourse.mybir as mybir
import concourse.tile as tile
from concourse.bass_utils import run_bass_kernel_spmd

D = 1024
DFF = 4096
E = 8
TOP_K = 2
P = 128
KD = D // P      # 8  contraction tiles for GEMM1
NF = DFF // P    # 32 dff tiles (GEMM1 out / GEMM2 contraction)
ND = D // P      # 8  GEMM2 out tiles
T_TILE = 512
# w1 f-chunks per DMA group; f-tile f lives in group G_OF[f] at within-group
# index J_OF[f]. (Uniform 4: every group is 128 DMA rows regardless of width,
# and early DMA is row-bound, so smaller first groups don't speed the head.)
FGS = [4, 4, 4, 4, 4, 4, 4, 4]
NG = len(FGS)
FG_OFF = [sum(FGS[:i]) for i in range(NG)]
G_OF, J_OF = [], []
for _g, _fg in enumerate(FGS):
    for _j in range(_fg):
        G_OF.append(_g)
        J_OF.append(_j)
WJ = 8           # w2 f-strips per tile (16KB DMA rows)
NW = NF // WJ    # 4 w2 tiles

BF16 = mybir.dt.bfloat16
F32 = mybir.dt.float32
NP_BF16 = np.dtype(ml_dtypes.bfloat16)

_neff_cache = {}


def _t_tiles(C):
    """Token tiles: 512s first, then split the remainder so no tile < 128."""
    out, rem = [], C
    while rem >= 512 + 128:
        out.append(512)
        rem -= 512
    if rem > 512:
        out.append(rem - 128)
        out.append(128)
    elif rem:
        out.append(rem)
    return out


def _split_multiwait_json(bir_bytes: bytes) -> bytes:
    """The walrus build in this container rejects instructions carrying more
    than one sync wait (or update). Split extras onto adjacent single-wait
    EventSemaphore carriers on the same engine: program order on the engine
    preserves the semantics exactly."""
    import json as _json

    bir = _json.loads(bir_bytes)
    for fn in bir["functions"]:
        for blk in fn["blocks"]:
            insts = blk.get("instructions", [])
            out = []
            for inst in insts:
                si = inst.get("sync_info")
                if si:
                    waits = si.get("on_wait") or []
                    if len(waits) > 1:
                        for i, w in enumerate(waits[:-1]):
                            out.append({
                                "debug": inst.get("debug", 0),
                                "engine": inst["engine"],
                                "ins": [],
                                "name": f"{inst['name']}_w{i}",
                                "opcode": "EventSemaphore",
                                "outs": [],
                                "sync_info": {"on_update": [], "on_wait": [w]},
                            })
                        si["on_wait"] = [waits[-1]]
                out.append(inst)
                if si:
                    ups = si.get("on_update") or []
                    if len(ups) > 1:
                        for i, u in enumerate(ups[1:]):
                            out.append({
                                "debug": inst.get("debug", 0),
                                "engine": inst["engine"],
                                "ins": [],
                                "name": f"{inst['name']}_u{i}",
                                "opcode": "EventSemaphore",
                                "outs": [],
                                "sync_info": {"on_update": [u], "on_wait": []},
                            })
                        si["on_update"] = [ups[0]]
            blk["instructions"] = out
    return _json.dumps(bir).encode()


def _patch_to_json(nc: bass.Bass) -> bass.Bass:
    orig = nc.to_json_bytes
    nc.to_json_bytes = lambda: _split_multiwait_json(orig())
    return nc


def _build_bass(tiles) -> bass.Bass:
    """One expert's MLP in transposed layouts (token dim = free axis).

    DRAM input layouts (pre-blocked on host so each partition's span is
    contiguous and large; DMAs are band-split across partitions onto
    parallel HW-DGE queues):
      xs : flat bf16; tile ti's block is [P, KD*tsz] p-major, rows KD*tsz*2 B
      w1x: [NG * P, KD * FG * P] bf16; row g*P+p holds w1 f-group g (8KB)
      w2x: [NW * P, WJ * D] bf16; row j*P+p holds w2 f-strips j*WJ.. (16KB)
      b1 : [DFF] f32
    Output:
      y  : [ND * n_t * P, T_TILE] bf16, block (dd, ti) at rows (dd*n_t+ti)*P
    """
    nc = bass.Bass()
    n_t = len(tiles)
    x_elems = P * KD * sum(tiles)
    xs_h = nc.dram_tensor("xs", [x_elems], BF16, kind="ExternalInput")
    w1_h = nc.dram_tensor("w1", [P * KD * DFF], BF16, kind="ExternalInput")
    b1_h = nc.dram_tensor("b1", [DFF], F32, kind="ExternalInput")
    w2_h = nc.dram_tensor("w2", [NW * P, WJ * D], BF16, kind="ExternalInput")
    y_h = nc.dram_tensor("y", [ND * n_t * P, T_TILE], BF16, kind="ExternalOutput")

    with ExitStack() as ctx:
        tc = ctx.enter_context(tile.TileContext(nc))
        spool = ctx.enter_context(tc.tile_pool(name="s", bufs=1))
        wpool = ctx.enter_context(tc.tile_pool(name="w", bufs=1))
        xpool = ctx.enter_context(tc.tile_pool(name="x", bufs=1))
        hpool = ctx.enter_context(tc.tile_pool(name="h", bufs=1))
        bpool = ctx.enter_context(tc.tile_pool(name="b", bufs=1))
        ypool = ctx.enter_context(tc.tile_pool(name="y", bufs=3))
        ps1 = ctx.enter_context(tc.tile_pool(name="ps1", bufs=3, space="PSUM"))
        ps2 = ctx.enter_context(tc.tile_pool(name="ps2", bufs=1, space="PSUM"))

        # --- PE warmup: run scratch matmuls during the initial DMA wait so
        # the HAM clock gate opens (K=8/8) before the first real matmul.
        scr = spool.tile([P, P + T_TILE], BF16, name="scr")
        nc.gpsimd.memset(scr[:], 1.0)
        # Back-to-back writes to one PSUM bank self-serialize at ~390ns/MM
        # (drain before has_written clear), which is what we want: a steady
        # ~12.5us of PE busy that ends just as the critical DMA lands.
        wps = ps1.tile([P, T_TILE], F32, tag="ps1", name="wps")
        for _ in range(40):
            nc.tensor.matmul(wps[:], scr[:, :P], scr[:, P:],
                             start=True, stop=True)

        # --- DMA plan. One dma_start's rows already fan out over all 16 DMA
        # engines of the bank, and the framework throttles at 4 outstanding
        # DMAs (rotating slot semaphores, wait-on-reuse). So: emit ~0.5MB
        # half-tensor pieces on the sync bank in priority order — the slot
        # rotation then paces later waves behind earlier ones, keeping the
        # critical first wave (x tile 0 + w1 group 0) alone on the wire.
        def w1_tile(g, npc, eng=None):
            fg = FGS[g]
            t = wpool.tile([P, KD, fg * P], BF16, tag=f"w1_{g}", name=f"w1_{g}")
            off = P * KD * FG_OFF[g] * P
            bp = P // npc
            blk = KD * fg * P
            for b in range(npc):
                o = off + b * bp * blk
                (eng or nc.sync).dma_start(
                    t[b * bp:(b + 1) * bp, :, :],
                    w1_h[o:o + bp * blk].rearrange(
                        "(p kd m) -> p kd m", p=bp, kd=KD),
                )
            return t

        def x_tile(ti, off, tsz, npc):
            t = xpool.tile([P, KD, T_TILE], BF16, tag=f"x{ti}", name=f"x{ti}")
            bp = P // npc
            blk = KD * tsz
            for b in range(npc):
                o = off + b * bp * blk
                nc.sync.dma_start(
                    t[b * bp:(b + 1) * bp, :, :tsz],
                    xs_h[o:o + bp * blk].rearrange(
                        "(p kd c) -> p kd c", p=bp, kd=KD),
                )
            return t

        def w2_tile(j):
            t = wpool.tile([P, WJ * D], BF16, tag=f"w2_{j}", name=f"w2_{j}")
            for b in range(2):
                r0 = j * P + b * 64
                nc.sync.dma_start(t[b * 64:(b + 1) * 64, :], w2_h[r0:r0 + 64, :])
            return t

        x_off = [P * KD * sum(tiles[:i]) for i in range(n_t)]
        x_t = [None] * n_t
        w1_t = [None] * NG
        w2_t = [None] * NW
        # Critical wave: x0 halves on the sync bank, w1 g0 halves on the
        # scalar bank. Early DMA is row-count bound (~600-800ns/row/engine,
        # descriptor fetches pipeline across queues), so spreading the 256
        # critical rows over both banks' queues shortens the head.
        x_t[0] = x_tile(0, x_off[0], tiles[0], 2)
        w1_t[0] = w1_tile(0, 2, eng=nc.scalar)
        # then the rest of w1 (needed at ~6.8us per 4-f group during GEMM1 of
        # tile 0), then w2 (GEMM2 of tile 0 consumes strip j over a ~27us
        # window thanks to the f-outer loop), then x1/x2 (needed much later).
        for g in range(1, NG):
            w1_t[g] = w1_tile(g, 1)
        for j in range(NW):
            w2_t[j] = w2_tile(j)
        if n_t > 1:
            x_t[1] = x_tile(1, x_off[1], tiles[1], 2)
        for ti in range(2, n_t):
            x_t[ti] = x_tile(ti, x_off[ti], tiles[ti], 1)
        # b1 rides the scalar HW-DGE bank so it beats the first gelu. It is
        # pre-transposed on the host to [P, NF] so this DMA is 128 contiguous
        # 128B rows — a strided elementwise gather here floods the shared DMA
        # engines with 4-byte packets and starves the weight streams.
        b1_t = bpool.tile([P, NF], F32)
        nc.scalar.dma_start(b1_t[:], b1_h[:].rearrange("(p f) -> p f", p=P))

        gelu = mybir.ActivationFunctionType.Gelu

        def w2_ap(f, dd):
            return w2_t[f // WJ][:, (f % WJ) * D + dd * P:
                                 (f % WJ) * D + (dd + 1) * P]

        def y_out(dd, ti, tsz, pt2):
            y_t = ypool.tile([P, T_TILE], BF16, tag="y", name="yt")
            nc.vector.tensor_copy(y_t[:, :tsz], pt2[:, :tsz])
            r0 = (dd * n_t + ti) * P
            nc.sync.dma_start(y_h[r0:r0 + P, :tsz], y_t[:, :tsz])

        for ti, tsz in enumerate(tiles):
            h_t = [hpool.tile([P, T_TILE], BF16, tag=f"h{f}", name=f"h{f}")
                   for f in range(NF)]
            for f in range(NF):
                pt = ps1.tile([P, T_TILE], F32, tag="ps1", name="pt1")
                for k in range(KD):
                    nc.tensor.matmul(
                        pt[:, :tsz],
                        w1_t[G_OF[f]][:, k, J_OF[f] * P:(J_OF[f] + 1) * P],
                        x_t[ti][:, k, :tsz],
                        start=(k == 0),
                        stop=(k == KD - 1),
                    )
                nc.scalar.activation(
                    h_t[f][:, :tsz], pt[:, :tsz], gelu, bias=b1_t[:, f:f + 1]
                )
            if ti < n_t - 1:
                # f-outer, dd-inner in halves of 4 PSUM banks: w2 strip j is
                # then consumed over a ~27us window instead of the first 7us
                # of GEMM2, so tile-0 GEMM2 doesn't stall on w2 arrival.
                for half in range(2):
                    pts = [ps2.tile([P, T_TILE], F32, tag=f"ps2_{i}",
                                    name=f"pt2_{i}") for i in range(4)]
                    for f in range(NF):
                        for i in range(4):
                            nc.tensor.matmul(
                                pts[i][:, :tsz],
                                w2_ap(f, half * 4 + i),
                                h_t[f][:, :tsz],
                                start=(f == 0),
                                stop=(f == NF - 1),
                            )
                    for i in range(4):
                        y_out(half * 4 + i, ti, tsz, pts[i])
            else:
                # last tile: dd-outer so y write-backs stream out during the
                # loop and only one copy+DMA lands in the kernel tail.
                for dd in range(ND):
                    pt2 = ps2.tile([P, T_TILE], F32, tag=f"ps2_{dd % 4}",
                                   name="pt2")
                    for f in range(NF):
                        nc.tensor.matmul(
                            pt2[:, :tsz],
                            w2_ap(f, dd),
                            h_t[f][:, :tsz],
                            start=(f == 0),
                            stop=(f == NF - 1),
                        )
                    y_out(dd, ti, tsz, pt2)
    return _patch_to_json(nc)


def _route(xf: np.ndarray, gate_w: np.ndarray):
    """Top-2 gating identical to the reference (argmax ties -> lower index)."""
    N = xf.shape[0]
    logits = xf @ gate_w  # (N, E) f32
    rows = np.arange(N)
    i1 = logits.argmax(1)
    v1 = logits[rows, i1]
    masked = logits.copy()
    masked[rows, i1] = -np.inf
    i2 = masked.argmax(1)
    v2 = masked[rows, i2]
    # softmax over the two selected logits (v1 >= v2)
    e = np.exp((v2 - v1).astype(np.float32))
    wt1 = (1.0 / (1.0 + e)).astype(np.float32)
    wt2 = (e / (1.0 + e)).astype(np.float32)
    idx_e, wts_e = [], []
    for ex in range(E):
        s1 = np.nonzero(i1 == ex)[0]
        s2 = np.nonzero(i2 == ex)[0]
        idx_e.append(np.concatenate([s1, s2]))
        wts_e.append(np.concatenate([wt1[s1], wt2[s2]]).astype(np.float32))
    return idx_e, wts_e


def kernel(x, gate_w, w1, b1, w2, b2, _trace=False):
    B, T, D_ = x.shape
    N = B * T
    xf = np.ascontiguousarray(x.reshape(N, D_).astype(np.float32))
    idx_e, wts_e = _route(xf, gate_w.astype(np.float32))
    cnts = [len(i) for i in idx_e]
    C = max(P, int(math.ceil(max(cnts) / 4)) * 4)
    tiles = _t_tiles(C)
    n_t = len(tiles)

    key = tuple(tiles)
    if key in _neff_cache:
        nc = _neff_cache[key]
    else:
        nc = _build_bass(tiles)
        _neff_cache[key] = nc

    in_maps = []
    for ex in range(E):
        cnt = cnts[ex]
        xg = np.zeros((C, D), np.float32)
        if cnt:
            xg[:cnt] = xf[idx_e[ex]]
        xgT = np.ascontiguousarray(xg.T).astype(NP_BF16)  # (D, C)
        # per tile ti: block [P, KD*tsz] with row p = concat_kd xgT[kd*P+p, t0:t0+tsz]
        parts = []
        t0 = 0
        for tsz in tiles:
            blk = (
                xgT[:, t0:t0 + tsz]
                .reshape(KD, P, tsz)
                .transpose(1, 0, 2)
                .reshape(P, KD * tsz)
            )
            parts.append(blk.ravel())
            t0 += tsz
        xs = np.concatenate(parts)
        # w1x: flat, group g block [P, KD*FGS[g]*P]: row p = concat_kd of
        # w1[kd*P+p, FG_OFF[g]*P : (FG_OFF[g]+FGS[g])*P]
        w1b = w1[ex].astype(NP_BF16)
        w1parts = []
        for g in range(NG):
            c0 = FG_OFF[g] * P
            c1 = c0 + FGS[g] * P
            blk = (
                w1b[:, c0:c1]
                .reshape(KD, P, FGS[g] * P)
                .transpose(1, 0, 2)
                .reshape(P, KD * FGS[g] * P)
            )
            w1parts.append(blk.ravel())
        w1x = np.concatenate(w1parts)
        # w2x[j*P + p, fi*D + d] = w2[(j*WJ+fi)*P + p, d]
        w2x = (
            w2[ex]
            .reshape(NW, WJ, P, D)
            .transpose(0, 2, 1, 3)
            .reshape(NW * P, WJ * D)
        )
        in_maps.append({
            "xs": np.ascontiguousarray(xs),
            "w1": np.ascontiguousarray(w1x),
            "b1": np.ascontiguousarray(
                b1[ex].astype(np.float32).reshape(NF, P).T).ravel(),
            "w2": np.ascontiguousarray(w2x).astype(NP_BF16),
        })

    res = run_bass_kernel_spmd(nc, in_maps, core_ids=list(range(E)), trace=_trace)
    if _trace:
        print(f"HW exec time: {res.exec_time_ns} ns")

    out = np.zeros((N, D), np.float32)
    for ex in range(E):
        cnt = cnts[ex]
        if not cnt:
            continue
        yb = res.results[ex]["y"]  # [ND*n_t*P, T_TILE] bf16
        yb = np.asarray(yb).astype(np.float32)
        yt = np.empty((D, cnt), np.float32)
        t0 = 0
        for ti, tsz in enumerate(tiles):
            if t0 >= cnt:
                break
            w = min(tsz, cnt - t0)
            for dd in range(ND):
                r0 = (dd * n_t + ti) * P
                yt[dd * P:(dd + 1) * P, t0:t0 + w] = yb[r0:r0 + P, :w]
            t0 += tsz
        yv = yt.T + b2[ex][None, :].astype(np.float32)
        out[idx_e[ex]] += wts_e[ex][:, None] * yv
    return out.reshape(B, T, D_)
